# revision 11
# baseline (speedup 1.0000x reference)
"""ChartCover (vq_codebook) Trainium2 kernel.

Data-parallel over batch B across 8 NeuronCores; centers/stats replicated.

All PE work runs in bf16 (1 cycle/row, fast weight loads) with fp32-class
accuracy recovered by operand splitting. The host splits z = b1 + b2 and
the whitening-folded center matrix U[d,m] = 2*c[m,d]/sigma_d = U1 + U2
(each bf16, so 16-bit effective mantissa per side), and the PE
accumulates three chains in fp32 PSUM:
    p = b1^T@U1 + b2^T@U1 + b1^T@U2 - cnk
(the omitted b2@U2 term is ~7e-4 on d2 ~ 1e3, far below the argmin/mask
noise floor). cnk[m] = |c_m|^2 + 2*mu_w.c_m is delivered into PSUM by a
ones-matmul against a 4-row exact bf16 residual split. Then
d2 = zn - p with host-fed row norms zn = ||z_w||^2, dists = ACT
sqrt(-p + zn), argmin = DVE max8/max_index directly on PSUM, onehot =
DVE (iota == idx) in bf16, and segment sums accumulate onehot^T @ b1
in PSUM across all tiles. Masks are derived on the host from the
returned dists (same comparison as the reference). GPSIMD is unused
(its elementwise ops measured ~4us per [128,256] tile = 20x DVE).
"""

from contextlib import ExitStack

import numpy as np

import concourse.bacc as bacc
import concourse.tile as tile
from concourse import mybir
from concourse.bass_utils import run_bass_kernel_spmd

B, D, M = 131072, 512, 256
R = 32.0
TAU = 0.01
EPS = 1e-6
NCORES = 8
P = 128
NCH = D // P  # 4 contraction chunks
F32 = mybir.dt.float32
BF16 = mybir.dt.bfloat16
AF = mybir.ActivationFunctionType
OP = mybir.AluOpType


def build_program(BC):
    NT = BC // P
    assert NT <= P
    nc = bacc.Bacc("TRN2", target_bir_lowering=False, debug=False)

    zb1 = nc.dram_tensor("zb1", [BC, D], BF16, kind="ExternalInput").ap()
    zb2 = nc.dram_tensor("zb2", [BC, D], BF16, kind="ExternalInput").ap()
    zn_in = nc.dram_tensor("zn", [NT, P], F32, kind="ExternalInput").ap()
    u_1 = nc.dram_tensor("u_1", [NCH, P, M], BF16, kind="ExternalInput").ap()
    u_2 = nc.dram_tensor("u_2", [NCH, P, M], BF16, kind="ExternalInput").ap()
    cnrows = nc.dram_tensor("cnrows", [P, M], BF16, kind="ExternalInput").ap()
    onesw = nc.dram_tensor("onesw", [P, P], BF16, kind="ExternalInput").ap()
    iota_rep = nc.dram_tensor("iota_rep", [P, M], F32, kind="ExternalInput").ap()
    ident = nc.dram_tensor("ident", [P, P], F32, kind="ExternalInput").ap()

    dists_o = nc.dram_tensor("dists", [BC, M], F32, kind="ExternalOutput").ap()
    idx_o = nc.dram_tensor("hard_idx", [BC], mybir.dt.int32, kind="ExternalOutput").ap()
    sums_o = nc.dram_tensor("sums", [M, D], F32, kind="ExternalOutput").ap()

    with tile.TileContext(nc) as tc, ExitStack() as ctx:
        const = ctx.enter_context(tc.tile_pool(name="const", bufs=1))
        zb1_pool = ctx.enter_context(tc.tile_pool(name="zb1p", bufs=4))
        zw_pool = ctx.enter_context(tc.tile_pool(name="zw", bufs=3))
        ep_pool = ctx.enter_context(tc.tile_pool(name="ep", bufs=4))
        oh_pool = ctx.enter_context(tc.tile_pool(name="oh", bufs=4))
        small_pool = ctx.enter_context(tc.tile_pool(name="small", bufs=6))
        pd_pool = ctx.enter_context(tc.tile_pool(name="pd", bufs=4, space="PSUM"))
        acc_pool = ctx.enter_context(tc.tile_pool(name="acc", bufs=1, space="PSUM"))

        sb_u1 = const.tile([P, NCH * M], BF16, tag="u1")
        sb_u2 = const.tile([P, NCH * M], BF16, tag="u2")
        for c in range(NCH):
            nc.sync.dma_start(sb_u1[:, c * M:(c + 1) * M], u_1[c])
            nc.sync.dma_start(sb_u2[:, c * M:(c + 1) * M], u_2[c])
        sb_cnrows = const.tile([P, M], BF16, tag="cnrows")
        nc.sync.dma_start(sb_cnrows[:], cnrows[:])
        sb_ones = const.tile([P, P], BF16, tag="ones")
        nc.sync.dma_start(sb_ones[:], onesw[:])
        sb_iota = const.tile([P, M], F32, tag="iota")
        nc.sync.dma_start(sb_iota[:], iota_rep[:])
        sb_zn = const.tile([P, NT], F32, tag="znc")
        nc.sync.dma_start(sb_zn[:], zn_in.rearrange("t b -> b t"))
        idx_stage = const.tile([P, P], F32, tag="idxstage")

        sums_ps = [acc_pool.tile([P, D], F32, tag=f"sums{i}", name=f"sums_ps{i}")
                   for i in range(2)]

        G = 4
        assert NT % G == 0
        GB = G * P  # rows per group
        for g in range(NT // G):
            b1T = zw_pool.tile([P, NCH * GB], BF16, tag="b1T", name="b1T")
            b2T = zw_pool.tile([P, NCH * GB], BF16, tag="b2T", name="b2T")
            for c in range(NCH):
                nc.sync.dma_start(
                    b1T[:, c * GB:(c + 1) * GB],
                    zb1[g * GB:(g + 1) * GB, c * P:(c + 1) * P],
                    transpose=True,
                )
                nc.sync.dma_start(
                    b2T[:, c * GB:(c + 1) * GB],
                    zb2[g * GB:(g + 1) * GB, c * P:(c + 1) * P],
                    transpose=True,
                )
            for tt in range(G):
              t = g * G + tt
              z1 = zb1_pool.tile([P, D], BF16, tag="zb1", name="z1")
              nc.sync.dma_start(z1[:], zb1[t * P:(t + 1) * P, :])

              pd = pd_pool.tile([P, M], F32, tag="pd", name="pd")
              nc.tensor.matmul(
                  pd[:], sb_ones[:], sb_cnrows[:],
                  start=True, stop=False, skip_group_check=True,
              )
              for c in range(NCH):
                  sl = slice(c * GB + tt * P, c * GB + (tt + 1) * P)
                  ms = slice(c * M, (c + 1) * M)
                  last = c == NCH - 1
                  nc.tensor.matmul(
                      pd[:], b1T[:, sl], sb_u1[:, ms],
                      start=False, stop=False, skip_group_check=True,
                  )
                  nc.tensor.matmul(
                      pd[:], b2T[:, sl], sb_u1[:, ms],
                      start=False, stop=False, skip_group_check=True,
                  )
                  nc.tensor.matmul(
                      pd[:], b1T[:, sl], sb_u2[:, ms],
                      start=False, stop=last, skip_group_check=True,
                  )
              # epilogue
              maxv = small_pool.tile([P, 8], F32, tag="maxv", name="maxv")
              nc.vector.max(maxv[:], pd[:])
              idx8 = small_pool.tile([P, 8], mybir.dt.uint32, tag="idx8", name="idx8")
              nc.vector.max_index(idx8[:], maxv[:], pd[:])
              nc.vector.tensor_copy(idx_stage[:, t:t + 1], idx8[:, 0:1])
              dsb = ep_pool.tile([P, M], F32, tag="dist", name="dsb")
              nc.scalar.activation(
                  dsb[:], pd[:], AF.Sqrt, bias=sb_zn[:, t:t + 1], scale=-1.0
              )
              oh = oh_pool.tile([P, M], BF16, tag="oh", name="oh")
              nc.vector.tensor_scalar(
                  out=oh[:], in0=sb_iota[:], scalar1=idx_stage[:, t:t + 1],
                  scalar2=None, op0=OP.is_equal,
              )
              nc.sync.dma_start(dists_o[t * P:(t + 1) * P, :], dsb[:])
              for i in range(2):
                  nc.tensor.matmul(
                      sums_ps[i][:], oh[:, i * P:(i + 1) * P], z1[:],
                      start=(t == 0), stop=(t == NT - 1), skip_group_check=True,
                  )

        # hard_idx: transpose [P, NT] staging -> [NT, P], cast to int32, store
        ps_idx = pd_pool.tile([P, M], F32, tag="pd", name="ps_idx")
        idsf = const.tile([P, P], F32, tag="idf")
        nc.sync.dma_start(idsf[:], ident[:])
        nc.tensor.transpose(ps_idx[:, 0:P], idx_stage[:], idsf[:])
        idxT = const.tile([P, P], mybir.dt.int32, tag="idxT")
        nc.vector.tensor_copy(idxT[0:NT, :], ps_idx[0:NT, 0:P])
        nc.sync.dma_start(idx_o.rearrange("(t b) -> t b", b=P), idxT[0:NT, :])
        for i in range(2):
            scp = ep_pool.tile([P, D], F32, tag="sumcp", name="scp")
            nc.scalar.activation(scp[:], sums_ps[i][:], AF.Copy)
            nc.sync.dma_start(sums_o[i * P:(i + 1) * P, :], scp[:])

    nc.compile()
    return nc


def make_consts(centers, stats_mean, stats_var):
    import ml_dtypes

    centers = np.asarray(centers, dtype=np.float32)
    mean64 = np.asarray(stats_mean, dtype=np.float64)
    var64 = np.asarray(stats_var, dtype=np.float64)
    istd64 = 1.0 / np.sqrt(var64 + EPS)

    U = (2.0 * centers.T.astype(np.float64) * istd64[:, None]).astype(np.float32)
    u1 = U.astype(ml_dtypes.bfloat16)
    u2 = (U.astype(np.float64) - u1.astype(np.float64)).astype(np.float32).astype(
        ml_dtypes.bfloat16
    )

    cn = np.sum(centers.astype(np.float64) ** 2, axis=1)
    # z_w.c = z.(c/sigma) - k,  k_m = sum_d (mu_d * istd_d) * c_{m,d}
    k = np.einsum("d,md->m", mean64 * istd64, centers.astype(np.float64))
    cnk = cn + 2.0 * k  # d2 = zn + cnk - 2 z.u
    # 4-row exact residual split of -cnk into bf16 rows
    rows = np.zeros((P, M), ml_dtypes.bfloat16)
    resid = (-cnk).copy()
    for r in range(4):
        v = resid.astype(np.float32).astype(ml_dtypes.bfloat16)
        rows[r] = v
        resid = resid - v.astype(np.float64)
    consts = {
        "u_1": np.ascontiguousarray(u1.reshape(NCH, P, M)),
        "u_2": np.ascontiguousarray(u2.reshape(NCH, P, M)),
        "cnrows": rows,
        "onesw": np.ones((P, P), ml_dtypes.bfloat16),
        "iota_rep": np.broadcast_to(np.arange(M, dtype=np.float32), (P, M)).copy(),
        "ident": np.eye(P, dtype=np.float32),
    }
    return consts


_CACHE = {}


def _get_prog(BC):
    if BC not in _CACHE:
        _CACHE[BC] = build_program(BC)
    return _CACHE[BC]


def finalize(host_out, centers, stats_mean, stats_var):
    centers = np.asarray(centers, dtype=np.float32)
    hard_idx = host_out["hard_idx"].astype(np.int32)
    counts = np.bincount(hard_idx, minlength=M).astype(np.float64)
    mean64 = np.asarray(stats_mean, dtype=np.float64)
    istd64 = 1.0 / np.sqrt(np.asarray(stats_var, dtype=np.float64) + EPS)
    sums_w = (host_out["sums_raw"] - counts[:, None] * mean64[None, :]) * istd64[None, :]
    mean_i = (sums_w / np.maximum(counts, 1.0)[:, None]).astype(np.float32)
    has = counts > 0
    upd = (np.float32(1.0 - TAU) * centers + np.float32(TAU) * mean_i).astype(np.float32)
    new_centers = np.where(has[:, None], upd, centers)
    dists = host_out["dists"]
    masks = dists <= np.float32(R)
    return (dists, hard_idx, masks, new_centers)


def kernel(z, centers, stats_mean, stats_var, trace=False):
    import ml_dtypes

    z = np.ascontiguousarray(np.asarray(z, dtype=np.float32))
    BC = z.shape[0] // NCORES
    nc = _get_prog(BC)
    consts = make_consts(centers, stats_mean, stats_var)

    mean32 = np.asarray(stats_mean, np.float32)
    std32 = np.sqrt(np.asarray(stats_var, np.float32) + np.float32(EPS))
    zw = ((z - mean32) / std32).astype(np.float32)
    zn = np.einsum("bd,bd->b", zw.astype(np.float64), zw.astype(np.float64)).astype(np.float32)
    zb1 = z.astype(ml_dtypes.bfloat16)
    zb2 = (z - zb1.astype(np.float32)).astype(ml_dtypes.bfloat16)

    NT = BC // P
    in_maps = [
        dict(
            consts,
            zb1=zb1[i * BC:(i + 1) * BC],
            zb2=zb2[i * BC:(i + 1) * BC],
            zn=np.ascontiguousarray(zn[i * BC:(i + 1) * BC].reshape(NT, P)),
        )
        for i in range(NCORES)
    ]
    res = run_bass_kernel_spmd(nc, in_maps, core_ids=list(range(NCORES)), trace=trace)
    rs = res.results
    host_out = {
        "dists": np.concatenate([r["dists"] for r in rs], axis=0),
        "hard_idx": np.concatenate([r["hard_idx"] for r in rs], axis=0),
        "sums_raw": np.sum(
            np.stack([r["sums"] for r in rs]).astype(np.float64), axis=0
        ),
    }
    out = finalize(host_out, centers, stats_mean, stats_var)
    if trace:
        return out, res
    return out


# revision 12
# speedup vs baseline: 2.0828x; 2.0828x over previous
"""ChartCover (vq_codebook) Trainium2 kernel.

Data-parallel over batch B across 8 NeuronCores; centers/stats replicated.

All PE work runs in bf16 (1 cycle/row, fast weight loads) with fp32-class
accuracy recovered by operand splitting. The host splits z = b1 + b2 and
the whitening-folded center matrix U[d,m] = 2*c[m,d]/sigma_d = U1 + U2
(each bf16, so 16-bit effective mantissa per side), and the PE
accumulates three chains in fp32 PSUM:
    p = b1^T@U1 + b2^T@U1 + b1^T@U2 - cnk
(the omitted b2@U2 term is ~7e-4 on d2 ~ 1e3, far below the argmin/mask
noise floor). cnk[m] = |c_m|^2 + 2*mu_w.c_m is delivered into PSUM by a
ones-matmul against a 4-row exact bf16 residual split. Then
d2 = zn - p with host-fed row norms zn = ||z_w||^2, dists = ACT
sqrt(-p + zn), argmin = DVE max8/max_index directly on PSUM, onehot =
DVE (iota == idx) in bf16, and segment sums accumulate onehot^T @ b1
in PSUM across all tiles. Masks are derived on the host from the
returned dists (same comparison as the reference). GPSIMD is unused
(its elementwise ops measured ~4us per [128,256] tile = 20x DVE).
"""

from contextlib import ExitStack

import numpy as np

import concourse.bacc as bacc
import concourse.tile as tile
from concourse import mybir
from concourse.bass_utils import run_bass_kernel_spmd

B, D, M = 131072, 512, 256
R = 32.0
TAU = 0.01
EPS = 1e-6
NCORES = 8
P = 128
NCH = D // P  # 4 contraction chunks
F32 = mybir.dt.float32
BF16 = mybir.dt.bfloat16
AF = mybir.ActivationFunctionType
OP = mybir.AluOpType


def build_program(BC):
    NT = BC // P
    assert NT <= P
    nc = bacc.Bacc("TRN2", target_bir_lowering=False, debug=False)

    zb1 = nc.dram_tensor("zb1", [BC, D], BF16, kind="ExternalInput").ap()
    zb1t = nc.dram_tensor("zb1t", [NCH, P, BC], BF16, kind="ExternalInput").ap()
    zb2t = nc.dram_tensor("zb2t", [NCH, P, BC], BF16, kind="ExternalInput").ap()
    zn_in = nc.dram_tensor("zn", [NT, P], F32, kind="ExternalInput").ap()
    u_1 = nc.dram_tensor("u_1", [NCH, P, M], BF16, kind="ExternalInput").ap()
    u_2 = nc.dram_tensor("u_2", [NCH, P, M], BF16, kind="ExternalInput").ap()
    cnrows = nc.dram_tensor("cnrows", [P, M], BF16, kind="ExternalInput").ap()
    onesw = nc.dram_tensor("onesw", [P, P], BF16, kind="ExternalInput").ap()
    iota_rep = nc.dram_tensor("iota_rep", [P, M], F32, kind="ExternalInput").ap()
    ident = nc.dram_tensor("ident", [P, P], F32, kind="ExternalInput").ap()

    dists_o = nc.dram_tensor("dists", [BC, M], F32, kind="ExternalOutput").ap()
    idx_o = nc.dram_tensor("hard_idx", [BC], mybir.dt.int32, kind="ExternalOutput").ap()
    sums_o = nc.dram_tensor("sums", [M, D], F32, kind="ExternalOutput").ap()

    with tile.TileContext(nc) as tc, ExitStack() as ctx:
        const = ctx.enter_context(tc.tile_pool(name="const", bufs=1))
        zb1_pool = ctx.enter_context(tc.tile_pool(name="zb1p", bufs=4))
        zw_pool = ctx.enter_context(tc.tile_pool(name="zw", bufs=3))
        ep_pool = ctx.enter_context(tc.tile_pool(name="ep", bufs=4))
        oh_pool = ctx.enter_context(tc.tile_pool(name="oh", bufs=4))
        small_pool = ctx.enter_context(tc.tile_pool(name="small", bufs=6))
        pd_pool = ctx.enter_context(tc.tile_pool(name="pd", bufs=4, space="PSUM"))
        acc_pool = ctx.enter_context(tc.tile_pool(name="acc", bufs=1, space="PSUM"))

        sb_u1 = const.tile([P, NCH * M], BF16, tag="u1")
        sb_u2 = const.tile([P, NCH * M], BF16, tag="u2")
        for c in range(NCH):
            nc.sync.dma_start(sb_u1[:, c * M:(c + 1) * M], u_1[c])
            nc.sync.dma_start(sb_u2[:, c * M:(c + 1) * M], u_2[c])
        sb_cnrows = const.tile([P, M], BF16, tag="cnrows")
        nc.sync.dma_start(sb_cnrows[:], cnrows[:])
        sb_ones = const.tile([P, P], BF16, tag="ones")
        nc.sync.dma_start(sb_ones[:], onesw[:])
        sb_iota = const.tile([P, M], F32, tag="iota")
        nc.sync.dma_start(sb_iota[:], iota_rep[:])
        sb_zn = const.tile([P, NT], F32, tag="znc")
        nc.sync.dma_start(sb_zn[:], zn_in.rearrange("t b -> b t"))
        idx_stage = const.tile([P, P], F32, tag="idxstage")

        sums_ps = [acc_pool.tile([P, D], F32, tag=f"sums{i}", name=f"sums_ps{i}")
                   for i in range(2)]

        zt1v = zb1t.rearrange("c p b -> p c b")
        zt2v = zb2t.rearrange("c p b -> p c b")
        for t in range(NT):
              z1 = zb1_pool.tile([P, D], BF16, tag="zb1", name="z1")
              nc.sync.dma_start(z1[:], zb1[t * P:(t + 1) * P, :])
              b1T = zw_pool.tile([P, D], BF16, tag="b1T", name="b1T")
              nc.sync.dma_start(
                  b1T[:].rearrange("p (c b) -> p c b", c=NCH),
                  zt1v[:, :, t * P:(t + 1) * P],
              )
              b2T = zw_pool.tile([P, D], BF16, tag="b2T", name="b2T")
              nc.sync.dma_start(
                  b2T[:].rearrange("p (c b) -> p c b", c=NCH),
                  zt2v[:, :, t * P:(t + 1) * P],
              )

              pd = pd_pool.tile([P, M], F32, tag="pd", name="pd")
              nc.tensor.matmul(
                  pd[:], sb_ones[:], sb_cnrows[:],
                  start=True, stop=False, skip_group_check=True,
              )
              for c in range(NCH):
                  sl = slice(c * P, (c + 1) * P)
                  ms = slice(c * M, (c + 1) * M)
                  last = c == NCH - 1
                  nc.tensor.matmul(
                      pd[:], b1T[:, sl], sb_u1[:, ms],
                      start=False, stop=False, skip_group_check=True,
                  )
                  nc.tensor.matmul(
                      pd[:], b2T[:, sl], sb_u1[:, ms],
                      start=False, stop=False, skip_group_check=True,
                  )
                  nc.tensor.matmul(
                      pd[:], b1T[:, sl], sb_u2[:, ms],
                      start=False, stop=last, skip_group_check=True,
                  )
              # epilogue
              maxv = small_pool.tile([P, 8], F32, tag="maxv", name="maxv")
              nc.vector.max(maxv[:], pd[:])
              idx8 = small_pool.tile([P, 8], mybir.dt.uint32, tag="idx8", name="idx8")
              nc.vector.max_index(idx8[:], maxv[:], pd[:])
              nc.vector.tensor_copy(idx_stage[:, t:t + 1], idx8[:, 0:1])
              dsb = ep_pool.tile([P, M], F32, tag="dist", name="dsb")
              nc.scalar.activation(
                  dsb[:], pd[:], AF.Sqrt, bias=sb_zn[:, t:t + 1], scale=-1.0
              )
              oh = oh_pool.tile([P, M], BF16, tag="oh", name="oh")
              nc.vector.tensor_scalar(
                  out=oh[:], in0=sb_iota[:], scalar1=idx_stage[:, t:t + 1],
                  scalar2=None, op0=OP.is_equal,
              )
              nc.sync.dma_start(dists_o[t * P:(t + 1) * P, :], dsb[:])
              for i in range(2):
                  nc.tensor.matmul(
                      sums_ps[i][:], oh[:, i * P:(i + 1) * P], z1[:],
                      start=(t == 0), stop=(t == NT - 1), skip_group_check=True,
                  )

        # hard_idx: transpose [P, NT] staging -> [NT, P], cast to int32, store
        ps_idx = pd_pool.tile([P, M], F32, tag="pd", name="ps_idx")
        idsf = const.tile([P, P], F32, tag="idf")
        nc.sync.dma_start(idsf[:], ident[:])
        nc.tensor.transpose(ps_idx[:, 0:P], idx_stage[:], idsf[:])
        idxT = const.tile([P, P], mybir.dt.int32, tag="idxT")
        nc.vector.tensor_copy(idxT[0:NT, :], ps_idx[0:NT, 0:P])
        nc.sync.dma_start(idx_o.rearrange("(t b) -> t b", b=P), idxT[0:NT, :])
        for i in range(2):
            scp = ep_pool.tile([P, D], F32, tag="sumcp", name="scp")
            nc.scalar.activation(scp[:], sums_ps[i][:], AF.Copy)
            nc.sync.dma_start(sums_o[i * P:(i + 1) * P, :], scp[:])

    nc.compile()
    return nc


def make_consts(centers, stats_mean, stats_var):
    import ml_dtypes

    centers = np.asarray(centers, dtype=np.float32)
    mean64 = np.asarray(stats_mean, dtype=np.float64)
    var64 = np.asarray(stats_var, dtype=np.float64)
    istd64 = 1.0 / np.sqrt(var64 + EPS)

    U = (2.0 * centers.T.astype(np.float64) * istd64[:, None]).astype(np.float32)
    u1 = U.astype(ml_dtypes.bfloat16)
    u2 = (U.astype(np.float64) - u1.astype(np.float64)).astype(np.float32).astype(
        ml_dtypes.bfloat16
    )

    cn = np.sum(centers.astype(np.float64) ** 2, axis=1)
    # z_w.c = z.(c/sigma) - k,  k_m = sum_d (mu_d * istd_d) * c_{m,d}
    k = np.einsum("d,md->m", mean64 * istd64, centers.astype(np.float64))
    cnk = cn + 2.0 * k  # d2 = zn + cnk - 2 z.u
    # 4-row exact residual split of -cnk into bf16 rows
    rows = np.zeros((P, M), ml_dtypes.bfloat16)
    resid = (-cnk).copy()
    for r in range(4):
        v = resid.astype(np.float32).astype(ml_dtypes.bfloat16)
        rows[r] = v
        resid = resid - v.astype(np.float64)
    consts = {
        "u_1": np.ascontiguousarray(u1.reshape(NCH, P, M)),
        "u_2": np.ascontiguousarray(u2.reshape(NCH, P, M)),
        "cnrows": rows,
        "onesw": np.ones((P, P), ml_dtypes.bfloat16),
        "iota_rep": np.broadcast_to(np.arange(M, dtype=np.float32), (P, M)).copy(),
        "ident": np.eye(P, dtype=np.float32),
    }
    return consts


_CACHE = {}


def _get_prog(BC):
    if BC not in _CACHE:
        _CACHE[BC] = build_program(BC)
    return _CACHE[BC]


def finalize(host_out, centers, stats_mean, stats_var):
    centers = np.asarray(centers, dtype=np.float32)
    hard_idx = host_out["hard_idx"].astype(np.int32)
    counts = np.bincount(hard_idx, minlength=M).astype(np.float64)
    mean64 = np.asarray(stats_mean, dtype=np.float64)
    istd64 = 1.0 / np.sqrt(np.asarray(stats_var, dtype=np.float64) + EPS)
    sums_w = (host_out["sums_raw"] - counts[:, None] * mean64[None, :]) * istd64[None, :]
    mean_i = (sums_w / np.maximum(counts, 1.0)[:, None]).astype(np.float32)
    has = counts > 0
    upd = (np.float32(1.0 - TAU) * centers + np.float32(TAU) * mean_i).astype(np.float32)
    new_centers = np.where(has[:, None], upd, centers)
    dists = host_out["dists"]
    masks = dists <= np.float32(R)
    return (dists, hard_idx, masks, new_centers)


def kernel(z, centers, stats_mean, stats_var, trace=False):
    import ml_dtypes

    z = np.ascontiguousarray(np.asarray(z, dtype=np.float32))
    BC = z.shape[0] // NCORES
    nc = _get_prog(BC)
    consts = make_consts(centers, stats_mean, stats_var)

    mean32 = np.asarray(stats_mean, np.float32)
    std32 = np.sqrt(np.asarray(stats_var, np.float32) + np.float32(EPS))
    zw = ((z - mean32) / std32).astype(np.float32)
    zn = np.einsum("bd,bd->b", zw.astype(np.float64), zw.astype(np.float64)).astype(np.float32)
    zb1 = z.astype(ml_dtypes.bfloat16)
    zb2 = (z - zb1.astype(np.float32)).astype(ml_dtypes.bfloat16)

    NT = BC // P
    in_maps = [
        dict(
            consts,
            zb1=zb1[i * BC:(i + 1) * BC],
            zb1t=np.ascontiguousarray(
                zb1[i * BC:(i + 1) * BC].T.reshape(NCH, P, BC)
            ),
            zb2t=np.ascontiguousarray(
                zb2[i * BC:(i + 1) * BC].T.reshape(NCH, P, BC)
            ),
            zn=np.ascontiguousarray(zn[i * BC:(i + 1) * BC].reshape(NT, P)),
        )
        for i in range(NCORES)
    ]
    res = run_bass_kernel_spmd(nc, in_maps, core_ids=list(range(NCORES)), trace=trace)
    rs = res.results
    host_out = {
        "dists": np.concatenate([r["dists"] for r in rs], axis=0),
        "hard_idx": np.concatenate([r["hard_idx"] for r in rs], axis=0),
        "sums_raw": np.sum(
            np.stack([r["sums"] for r in rs]).astype(np.float64), axis=0
        ),
    }
    out = finalize(host_out, centers, stats_mean, stats_var)
    if trace:
        return out, res
    return out


# revision 13
# speedup vs baseline: 2.2534x; 1.0819x over previous
"""ChartCover (vq_codebook) Trainium2 kernel.

Data-parallel over batch B across 8 NeuronCores; centers/stats replicated.

All PE work runs in bf16 (1 cycle/row, fast weight loads) with fp32-class
accuracy recovered by operand splitting. The host splits z = b1 + b2 and
the whitening-folded center matrix U[d,m] = 2*c[m,d]/sigma_d = U1 + U2
(each bf16, so 16-bit effective mantissa per side), and the PE
accumulates three chains in fp32 PSUM:
    p = b1^T@U1 + b2^T@U1 + b1^T@U2 - cnk
(the omitted b2@U2 term is ~7e-4 on d2 ~ 1e3, far below the argmin/mask
noise floor). cnk[m] = |c_m|^2 + 2*mu_w.c_m is delivered into PSUM by a
ones-matmul against a 4-row exact bf16 residual split. Then
d2 = zn - p with host-fed row norms zn = ||z_w||^2, dists = ACT
sqrt(-p + zn), argmin = DVE max8/max_index directly on PSUM, onehot =
DVE (iota == idx) in bf16, and segment sums accumulate onehot^T @ b1
in PSUM across all tiles. Masks are derived on the host from the
returned dists (same comparison as the reference). GPSIMD is unused
(its elementwise ops measured ~4us per [128,256] tile = 20x DVE).
"""

from contextlib import ExitStack

import numpy as np

import concourse.bacc as bacc
import concourse.tile as tile
from concourse import mybir
from concourse.bass_utils import run_bass_kernel_spmd

B, D, M = 131072, 512, 256
R = 32.0
TAU = 0.01
EPS = 1e-6
NCORES = 8
P = 128
NCH = D // P  # 4 contraction chunks
F32 = mybir.dt.float32
BF16 = mybir.dt.bfloat16
AF = mybir.ActivationFunctionType
OP = mybir.AluOpType


def build_program(BC):
    NT = BC // P
    assert NT <= P
    nc = bacc.Bacc("TRN2", target_bir_lowering=False, debug=False)

    zb1 = nc.dram_tensor("zb1", [BC, D], BF16, kind="ExternalInput").ap()
    zb1t = nc.dram_tensor("zb1t", [NT, P, D], BF16, kind="ExternalInput").ap()
    zb2t = nc.dram_tensor("zb2t", [NT, P, D], BF16, kind="ExternalInput").ap()
    zn_in = nc.dram_tensor("zn", [NT, P], F32, kind="ExternalInput").ap()
    u_1 = nc.dram_tensor("u_1", [NCH, P, M], BF16, kind="ExternalInput").ap()
    u_2 = nc.dram_tensor("u_2", [NCH, P, M], BF16, kind="ExternalInput").ap()
    cnrows = nc.dram_tensor("cnrows", [P, M], BF16, kind="ExternalInput").ap()
    onesw = nc.dram_tensor("onesw", [P, P], BF16, kind="ExternalInput").ap()
    iota_rep = nc.dram_tensor("iota_rep", [P, M], F32, kind="ExternalInput").ap()
    ident = nc.dram_tensor("ident", [P, P], F32, kind="ExternalInput").ap()

    dists_o = nc.dram_tensor("dists", [BC, M], F32, kind="ExternalOutput").ap()
    idx_o = nc.dram_tensor("hard_idx", [BC], mybir.dt.int32, kind="ExternalOutput").ap()
    sums_o = nc.dram_tensor("sums", [M, D], F32, kind="ExternalOutput").ap()

    with tile.TileContext(nc) as tc, ExitStack() as ctx:
        const = ctx.enter_context(tc.tile_pool(name="const", bufs=1))
        zb1_pool = ctx.enter_context(tc.tile_pool(name="zb1p", bufs=4))
        zw_pool = ctx.enter_context(tc.tile_pool(name="zw", bufs=3))
        ep_pool = ctx.enter_context(tc.tile_pool(name="ep", bufs=4))
        oh_pool = ctx.enter_context(tc.tile_pool(name="oh", bufs=4))
        small_pool = ctx.enter_context(tc.tile_pool(name="small", bufs=6))
        pd_pool = ctx.enter_context(tc.tile_pool(name="pd", bufs=4, space="PSUM"))
        acc_pool = ctx.enter_context(tc.tile_pool(name="acc", bufs=1, space="PSUM"))

        sb_u1 = const.tile([P, NCH * M], BF16, tag="u1")
        sb_u2 = const.tile([P, NCH * M], BF16, tag="u2")
        for c in range(NCH):
            nc.sync.dma_start(sb_u1[:, c * M:(c + 1) * M], u_1[c])
            nc.sync.dma_start(sb_u2[:, c * M:(c + 1) * M], u_2[c])
        sb_cnrows = const.tile([P, M], BF16, tag="cnrows")
        nc.sync.dma_start(sb_cnrows[:], cnrows[:])
        sb_ones = const.tile([P, P], BF16, tag="ones")
        nc.sync.dma_start(sb_ones[:], onesw[:])
        sb_iota = const.tile([P, M], F32, tag="iota")
        nc.sync.dma_start(sb_iota[:], iota_rep[:])
        sb_zn = const.tile([P, NT], F32, tag="znc")
        nc.sync.dma_start(sb_zn[:], zn_in.rearrange("t b -> b t"))
        idx_stage = const.tile([P, P], F32, tag="idxstage")

        sums_ps = [acc_pool.tile([P, D], F32, tag=f"sums{i}", name=f"sums_ps{i}")
                   for i in range(2)]

        for t in range(NT):
              z1 = zb1_pool.tile([P, D], BF16, tag="zb1", name="z1")
              nc.sync.dma_start(z1[:], zb1[t * P:(t + 1) * P, :])
              b1T = zw_pool.tile([P, D], BF16, tag="b1T", name="b1T")
              nc.sync.dma_start(b1T[:], zb1t[t])
              b2T = zw_pool.tile([P, D], BF16, tag="b2T", name="b2T")
              nc.sync.dma_start(b2T[:], zb2t[t])

              pd = pd_pool.tile([P, M], F32, tag="pd", name="pd")
              nc.tensor.matmul(
                  pd[:], sb_ones[:], sb_cnrows[:],
                  start=True, stop=False, skip_group_check=True,
              )
              for c in range(NCH):
                  sl = slice(c * P, (c + 1) * P)
                  ms = slice(c * M, (c + 1) * M)
                  last = c == NCH - 1
                  nc.tensor.matmul(
                      pd[:], b1T[:, sl], sb_u1[:, ms],
                      start=False, stop=False, skip_group_check=True,
                  )
                  nc.tensor.matmul(
                      pd[:], b2T[:, sl], sb_u1[:, ms],
                      start=False, stop=False, skip_group_check=True,
                  )
                  nc.tensor.matmul(
                      pd[:], b1T[:, sl], sb_u2[:, ms],
                      start=False, stop=last, skip_group_check=True,
                  )
              # epilogue
              maxv = small_pool.tile([P, 8], F32, tag="maxv", name="maxv")
              nc.vector.max(maxv[:], pd[:])
              idx8 = small_pool.tile([P, 8], mybir.dt.uint32, tag="idx8", name="idx8")
              nc.vector.max_index(idx8[:], maxv[:], pd[:])
              nc.vector.tensor_copy(idx_stage[:, t:t + 1], idx8[:, 0:1])
              dsb = ep_pool.tile([P, M], F32, tag="dist", name="dsb")
              nc.scalar.activation(
                  dsb[:], pd[:], AF.Sqrt, bias=sb_zn[:, t:t + 1], scale=-1.0
              )
              oh = oh_pool.tile([P, M], BF16, tag="oh", name="oh")
              nc.vector.tensor_scalar(
                  out=oh[:], in0=sb_iota[:], scalar1=idx_stage[:, t:t + 1],
                  scalar2=None, op0=OP.is_equal,
              )
              nc.sync.dma_start(dists_o[t * P:(t + 1) * P, :], dsb[:])
              for i in range(2):
                  nc.tensor.matmul(
                      sums_ps[i][:], oh[:, i * P:(i + 1) * P], z1[:],
                      start=(t == 0), stop=(t == NT - 1), skip_group_check=True,
                  )

        # hard_idx: transpose [P, NT] staging -> [NT, P], cast to int32, store
        ps_idx = pd_pool.tile([P, M], F32, tag="pd", name="ps_idx")
        idsf = const.tile([P, P], F32, tag="idf")
        nc.sync.dma_start(idsf[:], ident[:])
        nc.tensor.transpose(ps_idx[:, 0:P], idx_stage[:], idsf[:])
        idxT = const.tile([P, P], mybir.dt.int32, tag="idxT")
        nc.vector.tensor_copy(idxT[0:NT, :], ps_idx[0:NT, 0:P])
        nc.sync.dma_start(idx_o.rearrange("(t b) -> t b", b=P), idxT[0:NT, :])
        for i in range(2):
            scp = ep_pool.tile([P, D], F32, tag="sumcp", name="scp")
            nc.scalar.activation(scp[:], sums_ps[i][:], AF.Copy)
            nc.sync.dma_start(sums_o[i * P:(i + 1) * P, :], scp[:])

    nc.compile()
    return nc


def _tileT(shard):
    """[BC, D] -> [NT, P, D] where out[t, q, c*P + b] = shard[t*P + b, c*P + q]."""
    BC = shard.shape[0]
    NT = BC // P
    a = shard.reshape(NT, P, NCH, P).transpose(0, 3, 2, 1)
    return np.ascontiguousarray(a.reshape(NT, P, D))


def make_consts(centers, stats_mean, stats_var):
    import ml_dtypes

    centers = np.asarray(centers, dtype=np.float32)
    mean64 = np.asarray(stats_mean, dtype=np.float64)
    var64 = np.asarray(stats_var, dtype=np.float64)
    istd64 = 1.0 / np.sqrt(var64 + EPS)

    U = (2.0 * centers.T.astype(np.float64) * istd64[:, None]).astype(np.float32)
    u1 = U.astype(ml_dtypes.bfloat16)
    u2 = (U.astype(np.float64) - u1.astype(np.float64)).astype(np.float32).astype(
        ml_dtypes.bfloat16
    )

    cn = np.sum(centers.astype(np.float64) ** 2, axis=1)
    # z_w.c = z.(c/sigma) - k,  k_m = sum_d (mu_d * istd_d) * c_{m,d}
    k = np.einsum("d,md->m", mean64 * istd64, centers.astype(np.float64))
    cnk = cn + 2.0 * k  # d2 = zn + cnk - 2 z.u
    # 4-row exact residual split of -cnk into bf16 rows
    rows = np.zeros((P, M), ml_dtypes.bfloat16)
    resid = (-cnk).copy()
    for r in range(4):
        v = resid.astype(np.float32).astype(ml_dtypes.bfloat16)
        rows[r] = v
        resid = resid - v.astype(np.float64)
    consts = {
        "u_1": np.ascontiguousarray(u1.reshape(NCH, P, M)),
        "u_2": np.ascontiguousarray(u2.reshape(NCH, P, M)),
        "cnrows": rows,
        "onesw": np.ones((P, P), ml_dtypes.bfloat16),
        "iota_rep": np.broadcast_to(np.arange(M, dtype=np.float32), (P, M)).copy(),
        "ident": np.eye(P, dtype=np.float32),
    }
    return consts


_CACHE = {}


def _get_prog(BC):
    if BC not in _CACHE:
        _CACHE[BC] = build_program(BC)
    return _CACHE[BC]


def finalize(host_out, centers, stats_mean, stats_var):
    centers = np.asarray(centers, dtype=np.float32)
    hard_idx = host_out["hard_idx"].astype(np.int32)
    counts = np.bincount(hard_idx, minlength=M).astype(np.float64)
    mean64 = np.asarray(stats_mean, dtype=np.float64)
    istd64 = 1.0 / np.sqrt(np.asarray(stats_var, dtype=np.float64) + EPS)
    sums_w = (host_out["sums_raw"] - counts[:, None] * mean64[None, :]) * istd64[None, :]
    mean_i = (sums_w / np.maximum(counts, 1.0)[:, None]).astype(np.float32)
    has = counts > 0
    upd = (np.float32(1.0 - TAU) * centers + np.float32(TAU) * mean_i).astype(np.float32)
    new_centers = np.where(has[:, None], upd, centers)
    dists = host_out["dists"]
    masks = dists <= np.float32(R)
    return (dists, hard_idx, masks, new_centers)


def kernel(z, centers, stats_mean, stats_var, trace=False):
    import ml_dtypes

    z = np.ascontiguousarray(np.asarray(z, dtype=np.float32))
    BC = z.shape[0] // NCORES
    nc = _get_prog(BC)
    consts = make_consts(centers, stats_mean, stats_var)

    mean32 = np.asarray(stats_mean, np.float32)
    std32 = np.sqrt(np.asarray(stats_var, np.float32) + np.float32(EPS))
    zw = ((z - mean32) / std32).astype(np.float32)
    zn = np.einsum("bd,bd->b", zw.astype(np.float64), zw.astype(np.float64)).astype(np.float32)
    zb1 = z.astype(ml_dtypes.bfloat16)
    zb2 = (z - zb1.astype(np.float32)).astype(ml_dtypes.bfloat16)

    NT = BC // P
    in_maps = [
        dict(
            consts,
            zb1=zb1[i * BC:(i + 1) * BC],
            zb1t=_tileT(zb1[i * BC:(i + 1) * BC]),
            zb2t=_tileT(zb2[i * BC:(i + 1) * BC]),
            zn=np.ascontiguousarray(zn[i * BC:(i + 1) * BC].reshape(NT, P)),
        )
        for i in range(NCORES)
    ]
    res = run_bass_kernel_spmd(nc, in_maps, core_ids=list(range(NCORES)), trace=trace)
    rs = res.results
    host_out = {
        "dists": np.concatenate([r["dists"] for r in rs], axis=0),
        "hard_idx": np.concatenate([r["hard_idx"] for r in rs], axis=0),
        "sums_raw": np.sum(
            np.stack([r["sums"] for r in rs]).astype(np.float64), axis=0
        ),
    }
    out = finalize(host_out, centers, stats_mean, stats_var)
    if trace:
        return out, res
    return out


# revision 14
# speedup vs baseline: 2.9150x; 1.2936x over previous
"""ChartCover (vq_codebook) Trainium2 kernel.

Data-parallel over batch B across 8 NeuronCores; centers/stats replicated.

All PE work runs in bf16 (1 cycle/row, fast weight loads) with fp32-class
accuracy recovered by operand splitting. The host splits z = b1 + b2 and
the whitening-folded center matrix U[d,m] = 2*c[m,d]/sigma_d = U1 + U2
(each bf16, so 16-bit effective mantissa per side), and the PE
accumulates three chains in fp32 PSUM:
    p = b1^T@U1 + b2^T@U1 + b1^T@U2 - cnk
(the omitted b2@U2 term is ~7e-4 on d2 ~ 1e3, far below the argmin/mask
noise floor). cnk[m] = |c_m|^2 + 2*mu_w.c_m is delivered into PSUM by a
ones-matmul against a 4-row exact bf16 residual split. Then
d2 = zn - p with host-fed row norms zn = ||z_w||^2, dists = ACT
sqrt(-p + zn), argmin = DVE max8/max_index directly on PSUM, onehot =
DVE (iota == idx) in bf16, and segment sums accumulate onehot^T @ b1
in PSUM across all tiles. Masks are derived on the host from the
returned dists (same comparison as the reference). GPSIMD is unused
(its elementwise ops measured ~4us per [128,256] tile = 20x DVE).
"""

from contextlib import ExitStack

import numpy as np

import concourse.bacc as bacc
import concourse.tile as tile
from concourse import mybir
from concourse.bass_utils import run_bass_kernel_spmd

B, D, M = 131072, 512, 256
R = 32.0
TAU = 0.01
EPS = 1e-6
NCORES = 8
P = 128
NCH = D // P  # 4 contraction chunks
F32 = mybir.dt.float32
BF16 = mybir.dt.bfloat16
AF = mybir.ActivationFunctionType
OP = mybir.AluOpType


def build_program(BC):
    NT = BC // P
    assert NT <= P
    nc = bacc.Bacc("TRN2", target_bir_lowering=False, debug=False)

    zb1 = nc.dram_tensor("zb1", [BC, D], BF16, kind="ExternalInput").ap()
    zb1t = nc.dram_tensor("zb1t", [NT, P, D], BF16, kind="ExternalInput").ap()
    zb2t = nc.dram_tensor("zb2t", [NT, P, D], BF16, kind="ExternalInput").ap()
    zn_in = nc.dram_tensor("zn", [NT, P], F32, kind="ExternalInput").ap()
    u_1 = nc.dram_tensor("u_1", [NCH, P, M], BF16, kind="ExternalInput").ap()
    u_2 = nc.dram_tensor("u_2", [NCH, P, M], BF16, kind="ExternalInput").ap()
    cnrows = nc.dram_tensor("cnrows", [P, M], BF16, kind="ExternalInput").ap()
    onesw = nc.dram_tensor("onesw", [P, P], BF16, kind="ExternalInput").ap()
    iota_rep = nc.dram_tensor("iota_rep", [P, M], F32, kind="ExternalInput").ap()
    ident = nc.dram_tensor("ident", [P, P], F32, kind="ExternalInput").ap()

    dists_o = nc.dram_tensor("dists", [BC, M], F32, kind="ExternalOutput").ap()
    idx_o = nc.dram_tensor("hard_idx", [BC], mybir.dt.int32, kind="ExternalOutput").ap()
    sums_o = nc.dram_tensor("sums", [M, D], F32, kind="ExternalOutput").ap()

    with tile.TileContext(nc) as tc, ExitStack() as ctx:
        const = ctx.enter_context(tc.tile_pool(name="const", bufs=1))
        zb1_pool = ctx.enter_context(tc.tile_pool(name="zb1p", bufs=4))
        zw_pool = ctx.enter_context(tc.tile_pool(name="zw", bufs=3))
        ep_pool = ctx.enter_context(tc.tile_pool(name="ep", bufs=4))
        oh_pool = ctx.enter_context(tc.tile_pool(name="oh", bufs=4))
        small_pool = ctx.enter_context(tc.tile_pool(name="small", bufs=6))
        pd_pool = ctx.enter_context(tc.tile_pool(name="pd", bufs=4, space="PSUM"))
        acc_pool = ctx.enter_context(tc.tile_pool(name="acc", bufs=1, space="PSUM"))

        sb_u1 = const.tile([P, NCH * M], BF16, tag="u1")
        sb_u2 = const.tile([P, NCH * M], BF16, tag="u2")
        for c in range(NCH):
            nc.sync.dma_start(sb_u1[:, c * M:(c + 1) * M], u_1[c])
            nc.sync.dma_start(sb_u2[:, c * M:(c + 1) * M], u_2[c])
        sb_cnrows = const.tile([P, M], BF16, tag="cnrows")
        nc.sync.dma_start(sb_cnrows[:], cnrows[:])
        sb_ones = const.tile([P, P], BF16, tag="ones")
        nc.sync.dma_start(sb_ones[:], onesw[:])
        sb_iota = const.tile([P, M], F32, tag="iota")
        nc.sync.dma_start(sb_iota[:], iota_rep[:])
        sb_zn = const.tile([P, NT], F32, tag="znc")
        nc.sync.dma_start(sb_zn[:], zn_in.rearrange("t b -> b t"))
        idx_stage = const.tile([P, P], F32, tag="idxstage")

        sums_ps = [acc_pool.tile([P, D], F32, tag=f"sums{i}", name=f"sums_ps{i}")
                   for i in range(2)]

        G = 4
        assert NT % G == 0
        zb1v = zb1.rearrange("(t p) d -> p t d", p=P)
        dov = dists_o.rearrange("(t p) m -> p t m", p=P)
        for g in range(NT // G):
            gs = slice(g * G, (g + 1) * G)
            z1g = zb1_pool.tile([P, G * D], BF16, tag="zb1", name="z1g")
            nc.sync.dma_start(z1g[:].rearrange("p (t d) -> p t d", t=G), zb1v[:, gs])
            b1Tg = zw_pool.tile([P, G * D], BF16, tag="b1T", name="b1Tg")
            nc.sync.dma_start(
                b1Tg[:].rearrange("p (t d) -> p t d", t=G),
                zb1t[gs].rearrange("t p d -> p t d"),
            )
            b2Tg = zw_pool.tile([P, G * D], BF16, tag="b2T", name="b2Tg")
            nc.sync.dma_start(
                b2Tg[:].rearrange("p (t d) -> p t d", t=G),
                zb2t[gs].rearrange("t p d -> p t d"),
            )
            dstg = ep_pool.tile([P, G * M], F32, tag="dist", name="dstg")
            for tt in range(G):
              t = g * G + tt
              z1 = z1g[:, tt * D:(tt + 1) * D]
              b1T = b1Tg[:, tt * D:(tt + 1) * D]
              b2T = b2Tg[:, tt * D:(tt + 1) * D]

              pd = pd_pool.tile([P, M], F32, tag="pd", name="pd")
              nc.tensor.matmul(
                  pd[:], sb_ones[:], sb_cnrows[:],
                  start=True, stop=False, skip_group_check=True,
              )
              for c in range(NCH):
                  sl = slice(c * P, (c + 1) * P)
                  ms = slice(c * M, (c + 1) * M)
                  last = c == NCH - 1
                  nc.tensor.matmul(
                      pd[:], b1T[:, sl], sb_u1[:, ms],
                      start=False, stop=False, skip_group_check=True,
                  )
                  nc.tensor.matmul(
                      pd[:], b2T[:, sl], sb_u1[:, ms],
                      start=False, stop=False, skip_group_check=True,
                  )
                  nc.tensor.matmul(
                      pd[:], b1T[:, sl], sb_u2[:, ms],
                      start=False, stop=last, skip_group_check=True,
                  )
              # epilogue
              maxv = small_pool.tile([P, 8], F32, tag="maxv", name="maxv")
              nc.vector.max(maxv[:], pd[:])
              idx8 = small_pool.tile([P, 8], mybir.dt.uint32, tag="idx8", name="idx8")
              nc.vector.max_index(idx8[:], maxv[:], pd[:])
              nc.vector.tensor_copy(idx_stage[:, t:t + 1], idx8[:, 0:1])
              nc.scalar.activation(
                  dstg[:, tt * M:(tt + 1) * M], pd[:], AF.Sqrt,
                  bias=sb_zn[:, t:t + 1], scale=-1.0
              )
              oh = oh_pool.tile([P, M], BF16, tag="oh", name="oh")
              nc.vector.tensor_scalar(
                  out=oh[:], in0=sb_iota[:], scalar1=idx_stage[:, t:t + 1],
                  scalar2=None, op0=OP.is_equal,
              )
              for i in range(2):
                  nc.tensor.matmul(
                      sums_ps[i][:], oh[:, i * P:(i + 1) * P], z1,
                      start=(t == 0), stop=(t == NT - 1), skip_group_check=True,
                  )
            nc.sync.dma_start(
                dov[:, gs], dstg[:].rearrange("p (t m) -> p t m", t=G)
            )

        # hard_idx: transpose [P, NT] staging -> [NT, P], cast to int32, store
        ps_idx = pd_pool.tile([P, M], F32, tag="pd", name="ps_idx")
        idsf = const.tile([P, P], F32, tag="idf")
        nc.sync.dma_start(idsf[:], ident[:])
        nc.tensor.transpose(ps_idx[:, 0:P], idx_stage[:], idsf[:])
        idxT = const.tile([P, P], mybir.dt.int32, tag="idxT")
        nc.vector.tensor_copy(idxT[0:NT, :], ps_idx[0:NT, 0:P])
        nc.sync.dma_start(idx_o.rearrange("(t b) -> t b", b=P), idxT[0:NT, :])
        for i in range(2):
            scp = ep_pool.tile([P, D], F32, tag="sumcp", name="scp")
            nc.scalar.activation(scp[:], sums_ps[i][:], AF.Copy)
            nc.sync.dma_start(sums_o[i * P:(i + 1) * P, :], scp[:])

    nc.compile()
    return nc


def _tileT(shard):
    """[BC, D] -> [NT, P, D] where out[t, q, c*P + b] = shard[t*P + b, c*P + q]."""
    BC = shard.shape[0]
    NT = BC // P
    a = shard.reshape(NT, P, NCH, P).transpose(0, 3, 2, 1)
    return np.ascontiguousarray(a.reshape(NT, P, D))


def make_consts(centers, stats_mean, stats_var):
    import ml_dtypes

    centers = np.asarray(centers, dtype=np.float32)
    mean64 = np.asarray(stats_mean, dtype=np.float64)
    var64 = np.asarray(stats_var, dtype=np.float64)
    istd64 = 1.0 / np.sqrt(var64 + EPS)

    U = (2.0 * centers.T.astype(np.float64) * istd64[:, None]).astype(np.float32)
    u1 = U.astype(ml_dtypes.bfloat16)
    u2 = (U.astype(np.float64) - u1.astype(np.float64)).astype(np.float32).astype(
        ml_dtypes.bfloat16
    )

    cn = np.sum(centers.astype(np.float64) ** 2, axis=1)
    # z_w.c = z.(c/sigma) - k,  k_m = sum_d (mu_d * istd_d) * c_{m,d}
    k = np.einsum("d,md->m", mean64 * istd64, centers.astype(np.float64))
    cnk = cn + 2.0 * k  # d2 = zn + cnk - 2 z.u
    # 4-row exact residual split of -cnk into bf16 rows
    rows = np.zeros((P, M), ml_dtypes.bfloat16)
    resid = (-cnk).copy()
    for r in range(4):
        v = resid.astype(np.float32).astype(ml_dtypes.bfloat16)
        rows[r] = v
        resid = resid - v.astype(np.float64)
    consts = {
        "u_1": np.ascontiguousarray(u1.reshape(NCH, P, M)),
        "u_2": np.ascontiguousarray(u2.reshape(NCH, P, M)),
        "cnrows": rows,
        "onesw": np.ones((P, P), ml_dtypes.bfloat16),
        "iota_rep": np.broadcast_to(np.arange(M, dtype=np.float32), (P, M)).copy(),
        "ident": np.eye(P, dtype=np.float32),
    }
    return consts


_CACHE = {}


def _get_prog(BC):
    if BC not in _CACHE:
        _CACHE[BC] = build_program(BC)
    return _CACHE[BC]


def finalize(host_out, centers, stats_mean, stats_var):
    centers = np.asarray(centers, dtype=np.float32)
    hard_idx = host_out["hard_idx"].astype(np.int32)
    counts = np.bincount(hard_idx, minlength=M).astype(np.float64)
    mean64 = np.asarray(stats_mean, dtype=np.float64)
    istd64 = 1.0 / np.sqrt(np.asarray(stats_var, dtype=np.float64) + EPS)
    sums_w = (host_out["sums_raw"] - counts[:, None] * mean64[None, :]) * istd64[None, :]
    mean_i = (sums_w / np.maximum(counts, 1.0)[:, None]).astype(np.float32)
    has = counts > 0
    upd = (np.float32(1.0 - TAU) * centers + np.float32(TAU) * mean_i).astype(np.float32)
    new_centers = np.where(has[:, None], upd, centers)
    dists = host_out["dists"]
    masks = dists <= np.float32(R)
    return (dists, hard_idx, masks, new_centers)


def kernel(z, centers, stats_mean, stats_var, trace=False):
    import ml_dtypes

    z = np.ascontiguousarray(np.asarray(z, dtype=np.float32))
    BC = z.shape[0] // NCORES
    nc = _get_prog(BC)
    consts = make_consts(centers, stats_mean, stats_var)

    mean32 = np.asarray(stats_mean, np.float32)
    std32 = np.sqrt(np.asarray(stats_var, np.float32) + np.float32(EPS))
    zw = ((z - mean32) / std32).astype(np.float32)
    zn = np.einsum("bd,bd->b", zw.astype(np.float64), zw.astype(np.float64)).astype(np.float32)
    zb1 = z.astype(ml_dtypes.bfloat16)
    zb2 = (z - zb1.astype(np.float32)).astype(ml_dtypes.bfloat16)

    NT = BC // P
    in_maps = [
        dict(
            consts,
            zb1=zb1[i * BC:(i + 1) * BC],
            zb1t=_tileT(zb1[i * BC:(i + 1) * BC]),
            zb2t=_tileT(zb2[i * BC:(i + 1) * BC]),
            zn=np.ascontiguousarray(zn[i * BC:(i + 1) * BC].reshape(NT, P)),
        )
        for i in range(NCORES)
    ]
    res = run_bass_kernel_spmd(nc, in_maps, core_ids=list(range(NCORES)), trace=trace)
    rs = res.results
    host_out = {
        "dists": np.concatenate([r["dists"] for r in rs], axis=0),
        "hard_idx": np.concatenate([r["hard_idx"] for r in rs], axis=0),
        "sums_raw": np.sum(
            np.stack([r["sums"] for r in rs]).astype(np.float64), axis=0
        ),
    }
    out = finalize(host_out, centers, stats_mean, stats_var)
    if trace:
        return out, res
    return out


# revision 16
# speedup vs baseline: 2.9884x; 1.0252x over previous
"""ChartCover (vq_codebook) Trainium2 kernel.

Data-parallel over batch B across 8 NeuronCores; centers/stats replicated.

All PE work runs in bf16 (1 cycle/row, fast weight loads) with fp32-class
accuracy recovered by operand splitting. The host splits z = b1 + b2 and
the whitening-folded center matrix U[d,m] = 2*c[m,d]/sigma_d = U1 + U2
(each bf16, so 16-bit effective mantissa per side), and the PE
accumulates three chains in fp32 PSUM:
    p = b1^T@U1 + b2^T@U1 + b1^T@U2 - cnk
(the omitted b2@U2 term is ~7e-4 on d2 ~ 1e3, far below the argmin/mask
noise floor). cnk[m] = |c_m|^2 + 2*mu_w.c_m is delivered into PSUM by a
ones-matmul against a 4-row exact bf16 residual split. Then
d2 = zn - p with host-fed row norms zn = ||z_w||^2, dists = ACT
sqrt(-p + zn), argmin = DVE max8/max_index directly on PSUM, onehot =
DVE (iota == idx) in bf16, and segment sums accumulate onehot^T @ b1
in PSUM across all tiles. Masks are derived on the host from the
returned dists (same comparison as the reference). GPSIMD is unused
(its elementwise ops measured ~4us per [128,256] tile = 20x DVE).
"""

from contextlib import ExitStack

import numpy as np

import concourse.bacc as bacc
import concourse.tile as tile
from concourse import mybir
from concourse.bass_utils import run_bass_kernel_spmd

B, D, M = 131072, 512, 256
R = 32.0
TAU = 0.01
EPS = 1e-6
NCORES = 8
P = 128
NCH = D // P  # 4 contraction chunks
F32 = mybir.dt.float32
BF16 = mybir.dt.bfloat16
AF = mybir.ActivationFunctionType
OP = mybir.AluOpType


def build_program(BC):
    NT = BC // P
    assert NT <= P
    nc = bacc.Bacc("TRN2", target_bir_lowering=False, debug=False)

    zb1 = nc.dram_tensor("zb1", [BC, D], BF16, kind="ExternalInput").ap()
    zb1t = nc.dram_tensor("zb1t", [NT, P, D], BF16, kind="ExternalInput").ap()
    zb2t = nc.dram_tensor("zb2t", [NT, P, D], BF16, kind="ExternalInput").ap()
    zn_in = nc.dram_tensor("zn", [NT, P], F32, kind="ExternalInput").ap()
    u_1 = nc.dram_tensor("u_1", [NCH, P, M], BF16, kind="ExternalInput").ap()
    u_2 = nc.dram_tensor("u_2", [NCH, P, M], BF16, kind="ExternalInput").ap()
    cnrows = nc.dram_tensor("cnrows", [P, M], BF16, kind="ExternalInput").ap()
    onesw = nc.dram_tensor("onesw", [P, P], BF16, kind="ExternalInput").ap()
    iota_rep = nc.dram_tensor("iota_rep", [P, M], F32, kind="ExternalInput").ap()
    ident = nc.dram_tensor("ident", [P, P], F32, kind="ExternalInput").ap()

    dists_o = nc.dram_tensor("dists", [BC, M], F32, kind="ExternalOutput").ap()
    idx_o = nc.dram_tensor("hard_idx", [BC], mybir.dt.int32, kind="ExternalOutput").ap()
    sums_o = nc.dram_tensor("sums", [M, D], F32, kind="ExternalOutput").ap()

    with tile.TileContext(nc) as tc, ExitStack() as ctx:
        const = ctx.enter_context(tc.tile_pool(name="const", bufs=1))
        zb1_pool = ctx.enter_context(tc.tile_pool(name="zb1p", bufs=3))
        zw_pool = ctx.enter_context(tc.tile_pool(name="zw", bufs=3))
        ep_pool = ctx.enter_context(tc.tile_pool(name="ep", bufs=4))
        oh_pool = ctx.enter_context(tc.tile_pool(name="oh", bufs=4))
        small_pool = ctx.enter_context(tc.tile_pool(name="small", bufs=6))
        pd_pool = ctx.enter_context(tc.tile_pool(name="pd", bufs=4, space="PSUM"))
        acc_pool = ctx.enter_context(tc.tile_pool(name="acc", bufs=1, space="PSUM"))

        sb_u1 = const.tile([P, NCH * M], BF16, tag="u1")
        sb_u2 = const.tile([P, NCH * M], BF16, tag="u2")
        for c in range(NCH):
            nc.scalar.dma_start(sb_u1[:, c * M:(c + 1) * M], u_1[c])
            nc.scalar.dma_start(sb_u2[:, c * M:(c + 1) * M], u_2[c])
        sb_cnrows = const.tile([P, M], BF16, tag="cnrows")
        nc.scalar.dma_start(sb_cnrows[:], cnrows[:])
        sb_ones = const.tile([P, P], BF16, tag="ones")
        nc.scalar.dma_start(sb_ones[:], onesw[:])
        sb_iota = const.tile([P, M], F32, tag="iota")
        nc.scalar.dma_start(sb_iota[:], iota_rep[:])
        sb_zn = const.tile([P, NT], F32, tag="znc")
        nc.scalar.dma_start(sb_zn[:], zn_in.rearrange("t b -> b t"))
        idx_stage = const.tile([P, P], F32, tag="idxstage")

        sums_ps = [acc_pool.tile([P, D], F32, tag=f"sums{i}", name=f"sums_ps{i}")
                   for i in range(2)]

        G = 4
        assert NT % G == 0
        zb1v = zb1.rearrange("(t p) d -> p t d", p=P)
        dov = dists_o.rearrange("(t p) m -> p t m", p=P)
        for g in range(NT // G):
            gs = slice(g * G, (g + 1) * G)
            z1g = zb1_pool.tile([P, G * D], BF16, tag="zb1", name="z1g")
            nc.sync.dma_start(z1g[:].rearrange("p (t d) -> p t d", t=G), zb1v[:, gs])
            b1Tg = zw_pool.tile([P, G * D], BF16, tag="b1T", name="b1Tg")
            nc.sync.dma_start(
                b1Tg[:].rearrange("p (t d) -> p t d", t=G),
                zb1t[gs].rearrange("t p d -> p t d"),
            )
            b2Tg = zw_pool.tile([P, G * D], BF16, tag="b2T", name="b2Tg")
            nc.sync.dma_start(
                b2Tg[:].rearrange("p (t d) -> p t d", t=G),
                zb2t[gs].rearrange("t p d -> p t d"),
            )
            dstg = ep_pool.tile([P, G * M], F32, tag="dist", name="dstg")
            for tt in range(G):
              t = g * G + tt
              z1 = z1g[:, tt * D:(tt + 1) * D]
              b1T = b1Tg[:, tt * D:(tt + 1) * D]
              b2T = b2Tg[:, tt * D:(tt + 1) * D]

              pd = pd_pool.tile([P, M], F32, tag="pd", name="pd")
              nc.tensor.matmul(
                  pd[:], sb_ones[:], sb_cnrows[:],
                  start=True, stop=False, skip_group_check=True,
              )
              for c in range(NCH):
                  sl = slice(c * P, (c + 1) * P)
                  ms = slice(c * M, (c + 1) * M)
                  last = c == NCH - 1
                  nc.tensor.matmul(
                      pd[:], b1T[:, sl], sb_u1[:, ms],
                      start=False, stop=False, skip_group_check=True,
                  )
                  nc.tensor.matmul(
                      pd[:], b2T[:, sl], sb_u1[:, ms],
                      start=False, stop=False, skip_group_check=True,
                  )
                  nc.tensor.matmul(
                      pd[:], b1T[:, sl], sb_u2[:, ms],
                      start=False, stop=last, skip_group_check=True,
                  )
              # epilogue
              maxv = small_pool.tile([P, 8], F32, tag="maxv", name="maxv")
              nc.vector.max(maxv[:], pd[:])
              idx8 = small_pool.tile([P, 8], mybir.dt.uint32, tag="idx8", name="idx8")
              nc.vector.max_index(idx8[:], maxv[:], pd[:])
              nc.vector.tensor_copy(idx_stage[:, t:t + 1], idx8[:, 0:1])
              nc.scalar.activation(
                  dstg[:, tt * M:(tt + 1) * M], pd[:], AF.Sqrt,
                  bias=sb_zn[:, t:t + 1], scale=-1.0
              )
              oh = oh_pool.tile([P, M], BF16, tag="oh", name="oh")
              nc.vector.tensor_scalar(
                  out=oh[:], in0=sb_iota[:], scalar1=idx_stage[:, t:t + 1],
                  scalar2=None, op0=OP.is_equal,
              )
              for i in range(2):
                  nc.tensor.matmul(
                      sums_ps[i][:], oh[:, i * P:(i + 1) * P], z1,
                      start=(t == 0), stop=(t == NT - 1), skip_group_check=True,
                  )
            nc.sync.dma_start(
                dov[:, gs], dstg[:].rearrange("p (t m) -> p t m", t=G)
            )

        # hard_idx: transpose [P, NT] staging -> [NT, P], cast to int32, store
        ps_idx = pd_pool.tile([P, M], F32, tag="pd", name="ps_idx")
        idsf = const.tile([P, P], F32, tag="idf")
        nc.sync.dma_start(idsf[:], ident[:])
        nc.tensor.transpose(ps_idx[:, 0:P], idx_stage[:], idsf[:])
        idxT = const.tile([P, P], mybir.dt.int32, tag="idxT")
        nc.vector.tensor_copy(idxT[0:NT, :], ps_idx[0:NT, 0:P])
        nc.sync.dma_start(idx_o.rearrange("(t b) -> t b", b=P), idxT[0:NT, :])
        for i in range(2):
            scp = ep_pool.tile([P, D], F32, tag="sumcp", name="scp")
            nc.scalar.activation(scp[:], sums_ps[i][:], AF.Copy)
            nc.sync.dma_start(sums_o[i * P:(i + 1) * P, :], scp[:])

    nc.compile()
    return nc


def _tileT(shard):
    """[BC, D] -> [NT, P, D] where out[t, q, c*P + b] = shard[t*P + b, c*P + q]."""
    BC = shard.shape[0]
    NT = BC // P
    a = shard.reshape(NT, P, NCH, P).transpose(0, 3, 2, 1)
    return np.ascontiguousarray(a.reshape(NT, P, D))


def make_consts(centers, stats_mean, stats_var):
    import ml_dtypes

    centers = np.asarray(centers, dtype=np.float32)
    mean64 = np.asarray(stats_mean, dtype=np.float64)
    var64 = np.asarray(stats_var, dtype=np.float64)
    istd64 = 1.0 / np.sqrt(var64 + EPS)

    U = (2.0 * centers.T.astype(np.float64) * istd64[:, None]).astype(np.float32)
    u1 = U.astype(ml_dtypes.bfloat16)
    u2 = (U.astype(np.float64) - u1.astype(np.float64)).astype(np.float32).astype(
        ml_dtypes.bfloat16
    )

    cn = np.sum(centers.astype(np.float64) ** 2, axis=1)
    # z_w.c = z.(c/sigma) - k,  k_m = sum_d (mu_d * istd_d) * c_{m,d}
    k = np.einsum("d,md->m", mean64 * istd64, centers.astype(np.float64))
    cnk = cn + 2.0 * k  # d2 = zn + cnk - 2 z.u
    # 4-row exact residual split of -cnk into bf16 rows
    rows = np.zeros((P, M), ml_dtypes.bfloat16)
    resid = (-cnk).copy()
    for r in range(4):
        v = resid.astype(np.float32).astype(ml_dtypes.bfloat16)
        rows[r] = v
        resid = resid - v.astype(np.float64)
    consts = {
        "u_1": np.ascontiguousarray(u1.reshape(NCH, P, M)),
        "u_2": np.ascontiguousarray(u2.reshape(NCH, P, M)),
        "cnrows": rows,
        "onesw": np.ones((P, P), ml_dtypes.bfloat16),
        "iota_rep": np.broadcast_to(np.arange(M, dtype=np.float32), (P, M)).copy(),
        "ident": np.eye(P, dtype=np.float32),
    }
    return consts


_CACHE = {}


def _get_prog(BC):
    if BC not in _CACHE:
        _CACHE[BC] = build_program(BC)
    return _CACHE[BC]


def finalize(host_out, centers, stats_mean, stats_var):
    centers = np.asarray(centers, dtype=np.float32)
    hard_idx = host_out["hard_idx"].astype(np.int32)
    counts = np.bincount(hard_idx, minlength=M).astype(np.float64)
    mean64 = np.asarray(stats_mean, dtype=np.float64)
    istd64 = 1.0 / np.sqrt(np.asarray(stats_var, dtype=np.float64) + EPS)
    sums_w = (host_out["sums_raw"] - counts[:, None] * mean64[None, :]) * istd64[None, :]
    mean_i = (sums_w / np.maximum(counts, 1.0)[:, None]).astype(np.float32)
    has = counts > 0
    upd = (np.float32(1.0 - TAU) * centers + np.float32(TAU) * mean_i).astype(np.float32)
    new_centers = np.where(has[:, None], upd, centers)
    dists = host_out["dists"]
    masks = dists <= np.float32(R)
    return (dists, hard_idx, masks, new_centers)


def kernel(z, centers, stats_mean, stats_var, trace=False):
    import ml_dtypes

    z = np.ascontiguousarray(np.asarray(z, dtype=np.float32))
    BC = z.shape[0] // NCORES
    nc = _get_prog(BC)
    consts = make_consts(centers, stats_mean, stats_var)

    mean32 = np.asarray(stats_mean, np.float32)
    std32 = np.sqrt(np.asarray(stats_var, np.float32) + np.float32(EPS))
    zw = ((z - mean32) / std32).astype(np.float32)
    zn = np.einsum("bd,bd->b", zw.astype(np.float64), zw.astype(np.float64)).astype(np.float32)
    zb1 = z.astype(ml_dtypes.bfloat16)
    zb2 = (z - zb1.astype(np.float32)).astype(ml_dtypes.bfloat16)

    NT = BC // P
    in_maps = [
        dict(
            consts,
            zb1=zb1[i * BC:(i + 1) * BC],
            zb1t=_tileT(zb1[i * BC:(i + 1) * BC]),
            zb2t=_tileT(zb2[i * BC:(i + 1) * BC]),
            zn=np.ascontiguousarray(zn[i * BC:(i + 1) * BC].reshape(NT, P)),
        )
        for i in range(NCORES)
    ]
    res = run_bass_kernel_spmd(nc, in_maps, core_ids=list(range(NCORES)), trace=trace)
    rs = res.results
    host_out = {
        "dists": np.concatenate([r["dists"] for r in rs], axis=0),
        "hard_idx": np.concatenate([r["hard_idx"] for r in rs], axis=0),
        "sums_raw": np.sum(
            np.stack([r["sums"] for r in rs]).astype(np.float64), axis=0
        ),
    }
    out = finalize(host_out, centers, stats_mean, stats_var)
    if trace:
        return out, res
    return out


# revision 17
# speedup vs baseline: 3.1195x; 1.0439x over previous
"""ChartCover (vq_codebook) Trainium2 kernel.

Data-parallel over batch B across 8 NeuronCores; centers/stats replicated.

All PE work runs in bf16 (1 cycle/row, fast weight loads) with fp32-class
accuracy recovered by operand splitting. The host splits z = b1 + b2 and
the whitening-folded center matrix U[d,m] = 2*c[m,d]/sigma_d = U1 + U2
(each bf16, so 16-bit effective mantissa per side), and the PE
accumulates three chains in fp32 PSUM:
    p = b1^T@U1 + b2^T@U1 + b1^T@U2 - cnk
(the omitted b2@U2 term is ~7e-4 on d2 ~ 1e3, far below the argmin/mask
noise floor). cnk[m] = |c_m|^2 + 2*mu_w.c_m is delivered into PSUM by a
ones-matmul against a 4-row exact bf16 residual split. Then
d2 = zn - p with host-fed row norms zn = ||z_w||^2, dists = ACT
sqrt(-p + zn), argmin = DVE max8/max_index directly on PSUM, onehot =
DVE (iota == idx) in bf16, and segment sums accumulate onehot^T @ b1
in PSUM across all tiles. Masks are derived on the host from the
returned dists (same comparison as the reference). GPSIMD is unused
(its elementwise ops measured ~4us per [128,256] tile = 20x DVE).
"""

from contextlib import ExitStack

import numpy as np

import concourse.bacc as bacc
import concourse.tile as tile
from concourse import mybir
from concourse.bass_utils import run_bass_kernel_spmd

B, D, M = 131072, 512, 256
R = 32.0
TAU = 0.01
EPS = 1e-6
NCORES = 8
P = 128
NCH = D // P  # 4 contraction chunks
F32 = mybir.dt.float32
BF16 = mybir.dt.bfloat16
AF = mybir.ActivationFunctionType
OP = mybir.AluOpType


def build_program(BC):
    NT = BC // P
    assert NT <= P
    nc = bacc.Bacc("TRN2", target_bir_lowering=False, debug=False)

    zb1 = nc.dram_tensor("zb1", [BC, D], BF16, kind="ExternalInput").ap()
    zb1t = nc.dram_tensor("zb1t", [NT, P, D], BF16, kind="ExternalInput").ap()
    zb2t = nc.dram_tensor("zb2t", [NT, P, D], BF16, kind="ExternalInput").ap()
    zn_in = nc.dram_tensor("zn", [P, NT], F32, kind="ExternalInput").ap()
    u_1 = nc.dram_tensor("u_1", [NCH, P, M], BF16, kind="ExternalInput").ap()
    u_2 = nc.dram_tensor("u_2", [NCH, P, M], BF16, kind="ExternalInput").ap()
    cnrows = nc.dram_tensor("cnrows", [P, M], BF16, kind="ExternalInput").ap()
    onesw = nc.dram_tensor("onesw", [P, P], BF16, kind="ExternalInput").ap()
    iota_rep = nc.dram_tensor("iota_rep", [P, M], F32, kind="ExternalInput").ap()
    ident = nc.dram_tensor("ident", [P, P], F32, kind="ExternalInput").ap()

    dists_o = nc.dram_tensor("dists", [BC, M], F32, kind="ExternalOutput").ap()
    idx_o = nc.dram_tensor("hard_idx", [BC], mybir.dt.int32, kind="ExternalOutput").ap()
    sums_o = nc.dram_tensor("sums", [M, D], F32, kind="ExternalOutput").ap()

    with tile.TileContext(nc) as tc, ExitStack() as ctx:
        const = ctx.enter_context(tc.tile_pool(name="const", bufs=1))
        zb1_pool = ctx.enter_context(tc.tile_pool(name="zb1p", bufs=3))
        zw_pool = ctx.enter_context(tc.tile_pool(name="zw", bufs=3))
        ep_pool = ctx.enter_context(tc.tile_pool(name="ep", bufs=4))
        oh_pool = ctx.enter_context(tc.tile_pool(name="oh", bufs=4))
        small_pool = ctx.enter_context(tc.tile_pool(name="small", bufs=6))
        pd_pool = ctx.enter_context(tc.tile_pool(name="pd", bufs=5, space="PSUM"))
        acc_pool = ctx.enter_context(tc.tile_pool(name="acc", bufs=1, space="PSUM"))

        sb_u1c = []
        sb_u2c = []
        for c in range(NCH):
            u1c = const.tile([P, M], BF16, tag=f"u1c{c}", name=f"u1c{c}")
            nc.scalar.dma_start(u1c[:], u_1[c])
            sb_u1c.append(u1c)
            u2c = const.tile([P, M], BF16, tag=f"u2c{c}", name=f"u2c{c}")
            nc.scalar.dma_start(u2c[:], u_2[c])
            sb_u2c.append(u2c)
        sb_cnrows = const.tile([P, M], BF16, tag="cnrows")
        nc.scalar.dma_start(sb_cnrows[:], cnrows[:])
        sb_ones = const.tile([P, P], BF16, tag="ones")
        nc.scalar.dma_start(sb_ones[:], onesw[:])
        sb_iota = const.tile([P, M], F32, tag="iota")
        nc.scalar.dma_start(sb_iota[:], iota_rep[:])
        sb_zn = const.tile([P, NT], F32, tag="znc")
        nc.scalar.dma_start(sb_zn[:], zn_in[:])
        idx_stage = const.tile([P, P], F32, tag="idxstage")

        sums_ps = [acc_pool.tile([P, D], F32, tag=f"sums{i}", name=f"sums_ps{i}")
                   for i in range(2)]

        G = 4
        assert NT % G == 0
        zb1v = zb1.rearrange("(t p) d -> p t d", p=P)
        dov = dists_o.rearrange("(t p) m -> p t m", p=P)
        for g in range(NT // G):
            gs = slice(g * G, (g + 1) * G)
            z1g = zb1_pool.tile([P, G * D], BF16, tag="zb1", name="z1g")
            nc.sync.dma_start(z1g[:].rearrange("p (t d) -> p t d", t=G), zb1v[:, gs])
            b1Tg = zw_pool.tile([P, G * D], BF16, tag="b1T", name="b1Tg")
            nc.sync.dma_start(
                b1Tg[:].rearrange("p (t d) -> p t d", t=G),
                zb1t[gs].rearrange("t p d -> p t d"),
            )
            b2Tg = zw_pool.tile([P, G * D], BF16, tag="b2T", name="b2Tg")
            nc.sync.dma_start(
                b2Tg[:].rearrange("p (t d) -> p t d", t=G),
                zb2t[gs].rearrange("t p d -> p t d"),
            )
            dstg = ep_pool.tile([P, G * M], F32, tag="dist", name="dstg")
            for tt in range(G):
              t = g * G + tt
              z1 = z1g[:, tt * D:(tt + 1) * D]
              b1T = b1Tg[:, tt * D:(tt + 1) * D]
              b2T = b2Tg[:, tt * D:(tt + 1) * D]

              pd = pd_pool.tile([P, M], F32, tag="pd", name="pd")
              nc.tensor.matmul(
                  pd[:], sb_ones[:], sb_cnrows[:],
                  start=True, stop=False, skip_group_check=True,
              )
              for c in range(NCH):
                  sl = slice(c * P, (c + 1) * P)
                  last = c == NCH - 1
                  nc.tensor.matmul(
                      pd[:], b1T[:, sl], sb_u1c[c][:],
                      start=False, stop=False, skip_group_check=True,
                  )
                  nc.tensor.matmul(
                      pd[:], b2T[:, sl], sb_u1c[c][:],
                      start=False, stop=False, skip_group_check=True,
                  )
                  nc.tensor.matmul(
                      pd[:], b1T[:, sl], sb_u2c[c][:],
                      start=False, stop=last, skip_group_check=True,
                  )
              # epilogue
              maxv = small_pool.tile([P, 8], F32, tag="maxv", name="maxv")
              nc.vector.max(maxv[:], pd[:])
              idx8 = small_pool.tile([P, 8], mybir.dt.uint32, tag="idx8", name="idx8")
              nc.vector.max_index(idx8[:], maxv[:], pd[:])
              nc.vector.tensor_copy(idx_stage[:, t:t + 1], idx8[:, 0:1])
              nc.scalar.activation(
                  dstg[:, tt * M:(tt + 1) * M], pd[:], AF.Sqrt,
                  bias=sb_zn[:, t:t + 1], scale=-1.0
              )
              oh = oh_pool.tile([P, M], BF16, tag="oh", name="oh")
              nc.vector.tensor_scalar(
                  out=oh[:], in0=sb_iota[:], scalar1=idx_stage[:, t:t + 1],
                  scalar2=None, op0=OP.is_equal,
              )
              for i in range(2):
                  nc.tensor.matmul(
                      sums_ps[i][:], oh[:, i * P:(i + 1) * P], z1,
                      start=(t == 0), stop=(t == NT - 1), skip_group_check=True,
                  )
            nc.sync.dma_start(
                dov[:, gs], dstg[:].rearrange("p (t m) -> p t m", t=G)
            )

        # hard_idx: transpose [P, NT] staging -> [NT, P], cast to int32, store
        ps_idx = pd_pool.tile([P, M], F32, tag="pd", name="ps_idx")
        idsf = const.tile([P, P], F32, tag="idf")
        nc.sync.dma_start(idsf[:], ident[:])
        nc.tensor.transpose(ps_idx[:, 0:P], idx_stage[:], idsf[:])
        idxT = const.tile([P, P], mybir.dt.int32, tag="idxT")
        nc.vector.tensor_copy(idxT[0:NT, :], ps_idx[0:NT, 0:P])
        nc.sync.dma_start(idx_o.rearrange("(t b) -> t b", b=P), idxT[0:NT, :])
        for i in range(2):
            scp = ep_pool.tile([P, D], F32, tag="sumcp", name="scp")
            nc.scalar.activation(scp[:], sums_ps[i][:], AF.Copy)
            nc.sync.dma_start(sums_o[i * P:(i + 1) * P, :], scp[:])

    nc.compile()
    return nc


def _tileT(shard):
    """[BC, D] -> [NT, P, D] where out[t, q, c*P + b] = shard[t*P + b, c*P + q]."""
    BC = shard.shape[0]
    NT = BC // P
    a = shard.reshape(NT, P, NCH, P).transpose(0, 3, 2, 1)
    return np.ascontiguousarray(a.reshape(NT, P, D))


def make_consts(centers, stats_mean, stats_var):
    import ml_dtypes

    centers = np.asarray(centers, dtype=np.float32)
    mean64 = np.asarray(stats_mean, dtype=np.float64)
    var64 = np.asarray(stats_var, dtype=np.float64)
    istd64 = 1.0 / np.sqrt(var64 + EPS)

    U = (2.0 * centers.T.astype(np.float64) * istd64[:, None]).astype(np.float32)
    u1 = U.astype(ml_dtypes.bfloat16)
    u2 = (U.astype(np.float64) - u1.astype(np.float64)).astype(np.float32).astype(
        ml_dtypes.bfloat16
    )

    cn = np.sum(centers.astype(np.float64) ** 2, axis=1)
    # z_w.c = z.(c/sigma) - k,  k_m = sum_d (mu_d * istd_d) * c_{m,d}
    k = np.einsum("d,md->m", mean64 * istd64, centers.astype(np.float64))
    cnk = cn + 2.0 * k  # d2 = zn + cnk - 2 z.u
    # 4-row exact residual split of -cnk into bf16 rows
    rows = np.zeros((P, M), ml_dtypes.bfloat16)
    resid = (-cnk).copy()
    for r in range(4):
        v = resid.astype(np.float32).astype(ml_dtypes.bfloat16)
        rows[r] = v
        resid = resid - v.astype(np.float64)
    consts = {
        "u_1": np.ascontiguousarray(u1.reshape(NCH, P, M)),
        "u_2": np.ascontiguousarray(u2.reshape(NCH, P, M)),
        "cnrows": rows,
        "onesw": np.ones((P, P), ml_dtypes.bfloat16),
        "iota_rep": np.broadcast_to(np.arange(M, dtype=np.float32), (P, M)).copy(),
        "ident": np.eye(P, dtype=np.float32),
    }
    return consts


_CACHE = {}


def _get_prog(BC):
    if BC not in _CACHE:
        _CACHE[BC] = build_program(BC)
    return _CACHE[BC]


def finalize(host_out, centers, stats_mean, stats_var):
    centers = np.asarray(centers, dtype=np.float32)
    hard_idx = host_out["hard_idx"].astype(np.int32)
    counts = np.bincount(hard_idx, minlength=M).astype(np.float64)
    mean64 = np.asarray(stats_mean, dtype=np.float64)
    istd64 = 1.0 / np.sqrt(np.asarray(stats_var, dtype=np.float64) + EPS)
    sums_w = (host_out["sums_raw"] - counts[:, None] * mean64[None, :]) * istd64[None, :]
    mean_i = (sums_w / np.maximum(counts, 1.0)[:, None]).astype(np.float32)
    has = counts > 0
    upd = (np.float32(1.0 - TAU) * centers + np.float32(TAU) * mean_i).astype(np.float32)
    new_centers = np.where(has[:, None], upd, centers)
    dists = host_out["dists"]
    masks = dists <= np.float32(R)
    return (dists, hard_idx, masks, new_centers)


def kernel(z, centers, stats_mean, stats_var, trace=False):
    import ml_dtypes

    z = np.ascontiguousarray(np.asarray(z, dtype=np.float32))
    BC = z.shape[0] // NCORES
    nc = _get_prog(BC)
    consts = make_consts(centers, stats_mean, stats_var)

    mean32 = np.asarray(stats_mean, np.float32)
    std32 = np.sqrt(np.asarray(stats_var, np.float32) + np.float32(EPS))
    zw = ((z - mean32) / std32).astype(np.float32)
    zn = np.einsum("bd,bd->b", zw.astype(np.float64), zw.astype(np.float64)).astype(np.float32)
    zb1 = z.astype(ml_dtypes.bfloat16)
    zb2 = (z - zb1.astype(np.float32)).astype(ml_dtypes.bfloat16)

    NT = BC // P
    in_maps = [
        dict(
            consts,
            zb1=zb1[i * BC:(i + 1) * BC],
            zb1t=_tileT(zb1[i * BC:(i + 1) * BC]),
            zb2t=_tileT(zb2[i * BC:(i + 1) * BC]),
            zn=np.ascontiguousarray(zn[i * BC:(i + 1) * BC].reshape(NT, P).T),
        )
        for i in range(NCORES)
    ]
    res = run_bass_kernel_spmd(nc, in_maps, core_ids=list(range(NCORES)), trace=trace)
    rs = res.results
    host_out = {
        "dists": np.concatenate([r["dists"] for r in rs], axis=0),
        "hard_idx": np.concatenate([r["hard_idx"] for r in rs], axis=0),
        "sums_raw": np.sum(
            np.stack([r["sums"] for r in rs]).astype(np.float64), axis=0
        ),
    }
    out = finalize(host_out, centers, stats_mean, stats_var)
    if trace:
        return out, res
    return out
